# revision 1
# baseline (speedup 1.0000x reference)
"""Trainium2 Bass kernel for CausalSelfAttention (B=2, T=2048, C=1024, H=16).

Sharding: 8 cores = 2 batches x 4 head-groups (4 heads each).
Per core: QKV proj (its heads) -> causal attention (S^T layout, fp16 attn
weights) -> AllToAll reshard heads->tokens -> output proj for a 512-token
slice + bias + residual dropout.
"""

import sys

sys.path.insert(0, "/opt/trn_rl_repo")

import numpy as np

import concourse.bass as bass
import concourse.mybir as mybir
import concourse.tile as tile
from concourse.tile import add_dep_helper
from concourse import bacc
from concourse.bass_utils import run_bass_kernel_spmd

F32 = mybir.dt.float32
F32R = mybir.dt.float32r
F16 = mybir.dt.float16

B, T, C, H = 2, 2048, 1024, 16
HD = C // H  # 64
N_CORES = 8
GROUPS = 4            # head groups (one per core within a batch)
HPG = H // GROUPS     # heads per group = 4
PAIRS = HPG // 2      # head pairs per core = 2
TSL = T // GROUPS     # token slice per core = 512
KEEP = 0.9
EXP_BIAS = -3.0       # exp(s - 3): cancels in normalization, avoids fp16 overflow

DEBUG_TAPS = False

NT512 = T // 512      # 4 tq tiles of 512
NT128 = T // 128      # 16 tk tiles of 128
KT = C // 128         # 8 contraction tiles


def build_kernel():
    nc = bacc.Bacc("TRN2", target_bir_lowering=False, debug=False,
                   num_devices=N_CORES)

    # ---- per-core DRAM I/O ----
    xT = nc.dram_tensor("xT", [C, T], F32R, kind="ExternalInput")
    wqk = nc.dram_tensor("wqk", [C, 512], F32R, kind="ExternalInput")
    wv = nc.dram_tensor("wv", [C, 256], F32R, kind="ExternalInput")
    vbias = nc.dram_tensor("vbias", [128, 256], F32, kind="ExternalInput")
    bqk = nc.dram_tensor("bqk", [4, 128, 1], F32, kind="ExternalInput")
    wproj = nc.dram_tensor("wproj", [C, 256], F32R, kind="ExternalInput")
    bpr = nc.dram_tensor("bpr", [2, 128, 1], F32, kind="ExternalInput")
    maskT = nc.dram_tensor("maskT", [HPG, T, T], F16, kind="ExternalInput")
    rmaskT = nc.dram_tensor("rmaskT", [256, T], F16, kind="ExternalInput")
    triu_in = nc.dram_tensor("triu", [128, 128], F16, kind="ExternalInput")
    out = nc.dram_tensor("out", [256, T], F32, kind="ExternalOutput")
    dbg = {}
    if DEBUG_TAPS:
        dbg["qT0"] = nc.dram_tensor("dbg_qT0", [128, T], F32, kind="ExternalOutput")
        dbg["kT0"] = nc.dram_tensor("dbg_kT0", [128, T], F32, kind="ExternalOutput")
        dbg["v"] = nc.dram_tensor("dbg_v", [128, NT128 * 256], F16, kind="ExternalOutput")
        dbg["yT0"] = nc.dram_tensor("dbg_yT0", [128, T], F32, kind="ExternalOutput")
        dbg["yT1"] = nc.dram_tensor("dbg_yT1", [128, T], F32, kind="ExternalOutput")
        dbg["ya0"] = nc.dram_tensor("dbg_ya0", [128, T], F32, kind="ExternalOutput")
        dbg["ar"] = nc.dram_tensor("dbg_ar", [128, 1024], F16, kind="ExternalOutput")
        dbg["ad"] = nc.dram_tensor("dbg_ad", [128, 1024], F16, kind="ExternalOutput")
        dbg["ytmp"] = nc.dram_tensor("dbg_ytmp", [128, 512], F32, kind="ExternalOutput")
        dbg["denA"] = nc.dram_tensor("dbg_denA", [1, 512], F32, kind="ExternalOutput")
        dbg["denB"] = nc.dram_tensor("dbg_denB", [1, 512], F32, kind="ExternalOutput")
        dbg["rA"] = nc.dram_tensor("dbg_rA", [1, 512], F16, kind="ExternalOutput")
        dbg["rB"] = nc.dram_tensor("dbg_rB", [1, 512], F16, kind="ExternalOutput")

    # internal DRAM for the AllGather (collect all heads' y^T per batch group)
    ag_in = nc.dram_tensor("ag_in", [HPG * HD, T], F32R)
    ag_out = nc.dram_tensor("ag_out", [C, T], F32R)

    with tile.TileContext(nc) as tc:
        _build_body(nc, tc, xT, wqk, wv, vbias, bqk, wproj, bpr, maskT,
                    rmaskT, triu_in, out, ag_in, ag_out, dbg)
    nc.compile()
    return nc


def _build_body(nc, tc, xT, wqk, wv, vbias, bqk, wproj, bpr, maskT,
                rmaskT, triu_in, out, ag_in, ag_out, dbg=None):
    from contextlib import ExitStack
    ctx = ExitStack()

    # ---- PSUM (8 banks, managed manually) ----
    psA = ctx.enter_context(nc.psum_tensor([128, 1024], F32))   # banks 0-1
    psB = ctx.enter_context(nc.psum_tensor([128, 1024], F32))   # banks 2-3
    ps_y = ctx.enter_context(nc.psum_tensor([128, 512], F32))   # bank 4
    ps_cs = ctx.enter_context(nc.psum_tensor([128, 512], F32))  # bank 5
    ps_r = ctx.enter_context(nc.psum_tensor([128, 512], F32))   # bank 6
    ps_o = ctx.enter_context(nc.psum_tensor([128, 512], F32))   # bank 7

    # ---- persistent SBUF ----
    big = ctx.enter_context(tc.tile_pool(name="big", bufs=1))
    qT_sb = [big.tile([128, T], F32R, name=f"qT{p}") for p in range(PAIRS)]
    kT_sb = [big.tile([128, T], F32R, name=f"kT{p}") for p in range(PAIRS)]
    v_sb = big.tile([128, NT128 * 256], F16, name="v")
    yT_sb = [big.tile([128, T], F32R, name=f"yT{p}") for p in range(PAIRS)]
    vbias_sb = big.tile([128, 256], F32, name="vbias")
    bqk_sb = [big.tile([128, 1], F32, name=f"bqk{m}") for m in range(4)]
    bpr_sb = [big.tile([128, 1], F32, name=f"bpr{m}") for m in range(2)]
    rmask_sb = [big.tile([128, T], F16, name=f"rm{m}") for m in range(2)]
    wproj_sb = [big.tile([128, 256], F32R, name=f"wp{k}") for k in range(KT)]
    triu_sb = big.tile([128, 128], F16, name="triu")
    ones_cs = big.tile([128, 1], F16, name="ones_cs")
    ones_r16 = big.tile([1, 64], F16, name="ones_r16")
    expb_sb = big.tile([128, 1], F32, name="expb")

    # ---- rotating SBUF pools ----
    mpool = ctx.enter_context(tc.tile_pool(name="mask", bufs=3))
    apool = ctx.enter_context(tc.tile_pool(name="araw", bufs=1))
    dpool = ctx.enter_context(tc.tile_pool(name="adrop", bufs=2))
    spool = ctx.enter_context(tc.tile_pool(name="small", bufs=2))

    # ---- load constants / inputs ----
    nc.sync.dma_start(vbias_sb[:], vbias[:, :])
    for m in range(4):
        nc.sync.dma_start(bqk_sb[m][:], bqk.ap()[m])
    for m in range(2):
        nc.sync.dma_start(bpr_sb[m][:], bpr.ap()[m])
        nc.sync.dma_start(rmask_sb[m][:], rmaskT[m * 128:(m + 1) * 128, :])
    for k in range(KT):
        nc.sync.dma_start(wproj_sb[k][:], wproj[k * 128:(k + 1) * 128, :])
    nc.sync.dma_start(triu_sb[:], triu_in[:, :])
    nc.vector.memset(ones_cs[:], 1.0)
    nc.vector.memset(ones_r16[:], 1.0)
    nc.vector.memset(expb_sb[:], EXP_BIAS)

    # phase-1-only tensors live in their own pool; space reused by y_all later
    xpool = tc.alloc_tile_pool(name="xpool", bufs=1)
    xT_sb = [xpool.tile([128, T], F32R, name=f"xT{k}") for k in range(KT)]
    wqk_sb = [xpool.tile([128, 512], F32R, name=f"wqk{k}") for k in range(KT)]
    wv_sb = [xpool.tile([128, 256], F32R, name=f"wv{k}") for k in range(KT)]
    for k in range(KT):
        nc.sync.dma_start(xT_sb[k][:], xT[k * 128:(k + 1) * 128, :])
        nc.sync.dma_start(wqk_sb[k][:], wqk[k * 128:(k + 1) * 128, :])
        nc.sync.dma_start(wv_sb[k][:], wv[k * 128:(k + 1) * 128, :])

    def f32r(ap):
        return ap  # tensors already declared float32r

    # ================= Phase 1: QKV projection =================
    # Q^T,K^T: out[feat, tok]; m: 0=q-pair0, 1=q-pair1, 2=k-pair0, 3=k-pair1
    for m in range(4):
        for k in range(KT):
            for n in range(NT512):
                ps = psA if n < 2 else psB
                tgt = ps[:, (n % 2) * 512:(n % 2 + 1) * 512]
                nc.tensor.matmul(
                    tgt,
                    f32r(wqk_sb[k][:, m * 128:(m + 1) * 128]),
                    f32r(xT_sb[k][:, n * 512:(n + 1) * 512]),
                    start=(k == 0), stop=(k == KT - 1))
        dest = qT_sb[m] if m < 2 else kT_sb[m - 2]
        scale = 1.0 / np.sqrt(HD) if m < 2 else 1.0
        for n in range(NT512):
            ps = psA if n < 2 else psB
            src = ps[:, (n % 2) * 512:(n % 2 + 1) * 512]
            nc.scalar.activation(
                dest[:, n * 512:(n + 1) * 512], src,
                mybir.ActivationFunctionType.Identity,
                bias=bqk_sb[m][:, 0:1], scale=scale)

    # V: natural layout [tok, vfeat], fp16, bias added via broadcast tile
    for n in range(NT128):
        for k in range(KT):
            nc.tensor.matmul(
                ps_y[:, 0:256],
                f32r(xT_sb[k][:, n * 128:(n + 1) * 128]),
                f32r(wv_sb[k][:, 0:256]),
                start=(k == 0), stop=(k == KT - 1))
        nc.vector.tensor_tensor(
            v_sb[:, n * 256:(n + 1) * 256], ps_y[:, 0:256], vbias_sb[:],
            mybir.AluOpType.add)

    # xT no longer needed: release its pool so y_all reuses the space
    xpool.release()
    wpool = ctx.enter_context(tc.tile_pool(name="wpool", bufs=1))
    yall_sb = [wpool.tile([128, T], F32R, name=f"ya{k}") for k in range(KT)]

    # ================= Phase 2: attention =================
    for p in range(PAIRS):
        for j in range(NT512):
            n_i = 4 * j + 4  # tk tiles needed (block-causal)
            for i in range(n_i):
                r = max(0, i - 4 * j)
                off = 128 * r
                w = 512 - off
                ps_s = psA if i % 2 == 0 else psB
                # --- S^T = K^T.T @ Q^T (two heads row-tiled) ---
                w_mm = max(w, 256)
                off_mm = 512 - w_mm
                for h in range(2):
                    nc.tensor.matmul(
                        ps_s[:, h * 512 + off_mm:h * 512 + 512],
                        f32r(kT_sb[p][h * 64:(h + 1) * 64,
                                      i * 128:(i + 1) * 128]),
                        f32r(qT_sb[p][h * 64:(h + 1) * 64,
                                      j * 512 + off_mm:(j + 1) * 512]),
                        start=True, stop=True)
                # --- exp (valid range only) ---
                a_raw = apool.tile([128, 1024], F16, tag="araw")
                for h in range(2):
                    nc.scalar.activation(
                        a_raw[:, h * 512 + off:h * 512 + 512],
                        ps_s[:, h * 512 + off:h * 512 + 512],
                        mybir.ActivationFunctionType.Exp,
                        bias=expb_sb[:, 0:1], scale=1.0)
                # --- causal triangle on diagonal blocks ---
                if i >= 4 * j:
                    for h in range(2):
                        sl = a_raw[:, h * 512 + off:h * 512 + off + 128]
                        nc.vector.tensor_tensor(sl, sl, triu_sb[:],
                                                mybir.AluOpType.mult)
                # --- dropout mask ---
                m_tile = mpool.tile([128, 1024], F16, tag="mask")
                for h in range(2):
                    nc.sync.dma_start(
                        m_tile[:, h * 512 + off:h * 512 + 512],
                        maskT[2 * p + h, i * 128:(i + 1) * 128,
                              j * 512 + off:(j + 1) * 512])
                a_drop = dpool.tile([128, 1024], F16, tag="adrop")
                ad_v = a_drop[:].rearrange("p (h q) -> p h q", h=2)[:, :, off:512]
                ar_v = a_raw[:].rearrange("p (h q) -> p h q", h=2)[:, :, off:512]
                mt_v = m_tile[:].rearrange("p (h q) -> p h q", h=2)[:, :, off:512]
                nc.vector.tensor_tensor(ad_v, ar_v, mt_v, mybir.AluOpType.mult)
                if dbg and p == 0 and j == 0 and i == 0:
                    nc.sync.dma_start(dbg["ar"][:, :], a_raw[:])
                    nc.sync.dma_start(dbg["ad"][:, :], a_drop[:])
                # --- colsum (denominator): head A -> ps_cs, head B -> ps_r ---
                for h in range(2):
                    cs_bank = ps_cs if h == 0 else ps_r
                    nc.tensor.matmul(
                        cs_bank[0:1, off:512],
                        ones_cs[:],
                        a_raw[:, h * 512 + off:h * 512 + 512],
                        start=(i == 0),
                        stop=(i == n_i - 1),
                        skip_group_check=True)
                # --- AV (head A -> bank ps_y, head B -> bank ps_o) ---
                for h in range(2):
                    av_bank = ps_y if h == 0 else ps_o
                    nc.tensor.matmul(
                        av_bank[64 * h:64 * h + 64, off:512],
                        v_sb[:, i * 256 + (2 * p + h) * 64:
                             i * 256 + (2 * p + h) * 64 + 64],
                        a_drop[:, h * 512 + off:h * 512 + 512],
                        start=(i == 0),
                        stop=(i == n_i - 1),
                        skip_group_check=True)
            # --- normalize: y / (0.9 * den) ---
            recipA = spool.tile([1, 512], F16, tag="recipA")
            recipB = spool.tile([1, 512], F16, tag="recipB")
            with nc.allow_low_precision(reason="fp16 recip of softmax denom"):
                nc.vector.reciprocal(recipA[:], ps_cs[0:1, :])
                nc.vector.reciprocal(recipB[:], ps_r[0:1, :])
            # broadcast 1/den into psA's two banks (independent single groups)
            nc.tensor.matmul(psA[0:64, 0:512], ones_r16[:], recipA[:],
                             start=True, stop=True, skip_group_check=True)
            nc.tensor.matmul(psA[64:128, 512:1024], ones_r16[:], recipB[:],
                             start=True, stop=True, skip_group_check=True)
            y_tmp = spool.tile([128, 512], F32, tag="ytmp")
            nc.scalar.mul(y_tmp[0:64, :], ps_y[0:64, :], 1.0 / KEEP)
            nc.scalar.mul(y_tmp[64:128, :], ps_o[64:128, :], 1.0 / KEEP)
            if dbg and p == 0 and j == 0:
                nc.sync.dma_start(dbg["ytmp"][:, :], y_tmp[:])
                nc.sync.dma_start(dbg["rA"][:, :], recipA[:])
                nc.sync.dma_start(dbg["rB"][:, :], recipB[:])
                denA_t = spool.tile([1, 512], F32, tag="denA")
                denB_t = spool.tile([1, 512], F32, tag="denB")
                nc.scalar.copy(denA_t[:], ps_cs[0:1, :])
                nc.scalar.copy(denB_t[:], ps_r[0:1, :])
                nc.sync.dma_start(dbg["denA"][:, :], denA_t[:])
                nc.sync.dma_start(dbg["denB"][:, :], denB_t[:])
            nc.vector.tensor_tensor(yT_sb[p][0:64, j * 512:(j + 1) * 512],
                                    y_tmp[0:64, :], psA[0:64, 0:512],
                                    mybir.AluOpType.mult)
            nc.vector.tensor_tensor(yT_sb[p][64:128, j * 512:(j + 1) * 512],
                                    y_tmp[64:128, :], psA[64:128, 512:1024],
                                    mybir.AluOpType.mult)

    if dbg:
        nc.sync.dma_start(dbg["qT0"][:, :].bitcast(F32R), qT_sb[0][:])
        nc.sync.dma_start(dbg["kT0"][:, :].bitcast(F32R), kT_sb[0][:])
        nc.sync.dma_start(dbg["v"][:, :], v_sb[:])
        nc.sync.dma_start(dbg["yT0"][:, :].bitcast(F32R), yT_sb[0][:])
        nc.sync.dma_start(dbg["yT1"][:, :].bitcast(F32R), yT_sb[1][:])

    # ================= Phase 3: AllGather (batch-group of 4) =================
    for p in range(PAIRS):
        nc.sync.dma_start(ag_in[128 * p:128 * p + 128, :], yT_sb[p][:])
    nc.gpsimd.collective_compute(
        "AllGather", mybir.AluOpType.bypass,
        replica_groups=[[0, 1, 2, 3], [4, 5, 6, 7]],
        ins=[ag_in.ap()], outs=[ag_out.ap()])
    for k in range(KT):
        nc.sync.dma_start(yall_sb[k][:], ag_out[k * 128:(k + 1) * 128, :])
    if dbg:
        nc.sync.dma_start(dbg["ya0"][:, :].bitcast(F32R), yall_sb[0][:])

    # ================= Phase 4: output projection (our 256 co rows) ========
    for m in range(2):
        for n in range(NT512):
            ps = ps_o if (m * NT512 + n) % 2 == 0 else ps_r
            for k in range(KT):
                nc.tensor.matmul(
                    ps[:],
                    f32r(wproj_sb[k][:, m * 128:(m + 1) * 128]),
                    f32r(yall_sb[k][:, n * 512:(n + 1) * 512]),
                    start=(k == 0), stop=(k == KT - 1))
            t_m = spool.tile([128, TSL], F32, tag="tproj")
            nc.scalar.activation(t_m[:], ps[:],
                                 mybir.ActivationFunctionType.Identity,
                                 bias=bpr_sb[m][:, 0:1], scale=1.0 / KEEP)
            o_m = spool.tile([128, TSL], F32, tag="oproj")
            nc.vector.tensor_tensor(
                o_m[:], t_m[:],
                rmask_sb[m][:, n * 512:(n + 1) * 512],
                mybir.AluOpType.mult)
            nc.sync.dma_start(out[m * 128:(m + 1) * 128,
                                  n * 512:(n + 1) * 512], o_m[:])

    ctx.close()


def prep_inputs(x, Wqkv, bqkv, Wproj, bproj, attn_drop_mask, resid_drop_mask):
    """Shard + lay out the full inputs for the 8 cores."""
    x = np.asarray(x, np.float32)
    Wqkv = np.asarray(Wqkv, np.float32)
    bqkv = np.asarray(bqkv, np.float32)
    Wproj = np.asarray(Wproj, np.float32)
    bproj = np.asarray(bproj, np.float32)
    attn_drop_mask = np.asarray(attn_drop_mask, bool)
    resid_drop_mask = np.asarray(resid_drop_mask, bool)

    tril = np.tril(np.ones((T, T), dtype=bool))
    triu128 = np.triu(np.ones((128, 128), np.float16))
    in_maps = []
    for core in range(N_CORES):
        b, g = divmod(core, GROUPS)
        cs = slice(g * 256, (g + 1) * 256)  # this group's feature rows
        wqk_c = np.concatenate([Wqkv[:, cs], Wqkv[:, 1024:2048][:, cs]],
                               axis=1)
        wv_c = np.ascontiguousarray(Wqkv[:, 2048:3072][:, cs])
        bq = (bqkv[0:1024][cs] / np.float32(np.sqrt(HD))).astype(np.float32)
        bk = bqkv[1024:2048][cs]
        bv = bqkv[2048:3072][cs]
        bqk_c = np.stack([bq[0:128], bq[128:256], bk[0:128], bk[128:256]])
        bqk_c = bqk_c.reshape(4, 128, 1)
        vbias_c = np.broadcast_to(bv, (128, 256)).copy()
        # combined causal & dropout mask, transposed to [tk, tq], fp16
        m = attn_drop_mask[b, g * HPG:(g + 1) * HPG] & tril
        maskT_c = np.ascontiguousarray(
            m.transpose(0, 2, 1)).astype(np.float16)
        rmaskT_c = np.ascontiguousarray(
            resid_drop_mask[b, :, cs].T).astype(np.float16)
        bpr_c = (bproj[cs] / KEEP).astype(np.float32).reshape(2, 128, 1)
        in_maps.append(dict(
            xT=np.ascontiguousarray(x[b].T),
            wqk=np.ascontiguousarray(wqk_c),
            wv=wv_c,
            vbias=vbias_c.astype(np.float32),
            bqk=bqk_c.astype(np.float32),
            wproj=np.ascontiguousarray(Wproj[:, cs]),
            bpr=bpr_c,
            maskT=maskT_c,
            rmaskT=rmaskT_c,
            triu=triu128,
        ))
    return in_maps


_NC_CACHE = {}


def _get_nc():
    if "nc" not in _NC_CACHE:
        _NC_CACHE["nc"] = build_kernel()
    return _NC_CACHE["nc"]


def kernel(trace=False, **inputs):
    nc = _get_nc()
    in_maps = prep_inputs(**inputs)
    res = run_bass_kernel_spmd(nc, in_maps, core_ids=list(range(N_CORES)),
                               trace=trace)
    y = np.empty((B, T, C), np.float32)
    for core in range(N_CORES):
        b, g = divmod(core, GROUPS)
        y[b, :, g * 256:(g + 1) * 256] = res.results[core]["out"].T
    kernel.last_result = res
    return y



# revision 5
# speedup vs baseline: 1.7053x; 1.7053x over previous
"""Trainium2 Bass kernel for CausalSelfAttention (B=2, T=2048, C=1024, H=16).

Sharding: 8 cores = 2 batches x 4 head-groups (4 heads each).
Per core, interleaved by 512-token slice n:
  QKV proj (fp16) for slice n -> causal attention for query slice j=n
  (S^T layout, fp16; exp on Act; dropout-mask mult + softmax-denominator
  accumulation on DVE) -> chunked fp16 AllGather of y^T slice across the
  4 cores of the batch -> output proj for the previous slice (256 feature
  rows each), one chunk behind so the collective is hidden.
"""

import sys

sys.path.insert(0, "/opt/trn_rl_repo")

import numpy as np
import ml_dtypes

import concourse.bass as bass
import concourse.mybir as mybir
import concourse.tile as tile
from concourse import bacc
from concourse.bass_utils import run_bass_kernel_spmd

F32 = mybir.dt.float32
F16 = mybir.dt.float16
F8 = mybir.dt.float8e4

B, T, C, H = 2, 2048, 1024, 16
HD = C // H  # 64
N_CORES = 8
GROUPS = 4            # head groups (one per core within a batch)
HPG = H // GROUPS     # heads per group = 4
PAIRS = HPG // 2      # head pairs per core = 2
KEEP = 0.9
EXP_BIAS = -3.0       # exp(s - 3): cancels in normalization, avoids overflow

NT512 = T // 512      # 4 token slices of 512
KT = C // 128         # 8 contraction tiles


def build_kernel():
    nc = bacc.Bacc("TRN2", target_bir_lowering=False, debug=False,
                   num_devices=N_CORES)

    # ---- per-core DRAM I/O ----
    xT = nc.dram_tensor("xT", [C, T], F16, kind="ExternalInput")
    wqk = nc.dram_tensor("wqk", [C, 512], F16, kind="ExternalInput")
    wv = nc.dram_tensor("wv", [C, 256], F16, kind="ExternalInput")
    vbias = nc.dram_tensor("vbias", [128, 256], F32, kind="ExternalInput")
    bqk = nc.dram_tensor("bqk", [4, 128, 1], F32, kind="ExternalInput")
    wproj = nc.dram_tensor("wproj", [C, 256], F16, kind="ExternalInput")
    bpr = nc.dram_tensor("bpr", [2, 128, 1], F32, kind="ExternalInput")
    maskT = nc.dram_tensor("maskT", [HPG, T, T], F8, kind="ExternalInput")
    rmaskT = nc.dram_tensor("rmaskT", [256, T], F16, kind="ExternalInput")
    triu_in = nc.dram_tensor("triu", [128, 128], F16, kind="ExternalInput")
    out = nc.dram_tensor("out", [256, T], F32, kind="ExternalOutput")

    # internal DRAM for the chunked AllGather (per 512-token slice)
    ag_in = [nc.dram_tensor(f"ag_in{j}", [PAIRS * 128, 512], F16)
             for j in range(NT512)]
    ag_out = [nc.dram_tensor(f"ag_out{j}", [C, 512], F16)
              for j in range(NT512)]

    with tile.TileContext(nc) as tc:
        _build_body(nc, tc, xT, wqk, wv, vbias, bqk, wproj, bpr, maskT,
                    rmaskT, triu_in, out, ag_in, ag_out)
    nc.compile()
    return nc


def _build_body(nc, tc, xT, wqk, wv, vbias, bqk, wproj, bpr, maskT,
                rmaskT, triu_in, out, ag_in, ag_out):
    from contextlib import ExitStack
    ctx = ExitStack()
    AF = mybir.ActivationFunctionType
    ALU = mybir.AluOpType

    # ---- PSUM (8 banks) ----
    psA = ctx.enter_context(nc.psum_tensor([128, 1024], F32))   # S ping
    psB = ctx.enter_context(nc.psum_tensor([128, 1024], F32))   # S pong
    ps_y = ctx.enter_context(nc.psum_tensor([128, 512], F32))   # AV head A
    ps_o = ctx.enter_context(nc.psum_tensor([128, 512], F32))   # AV head B
    ps_u = ctx.enter_context(nc.psum_tensor([128, 512], F32))   # util ping
    ps_w = ctx.enter_context(nc.psum_tensor([128, 512], F32))   # util pong

    # ---- persistent SBUF ----
    big = ctx.enter_context(tc.tile_pool(name="big", bufs=1))
    qT_sb = [big.tile([128, T], F16, name=f"qT{p}") for p in range(PAIRS)]
    kT_sb = [big.tile([128, T], F16, name=f"kT{p}") for p in range(PAIRS)]
    v_sb = big.tile([128, (T // 128) * 256], F16, name="v")
    xT_sb = [big.tile([128, T], F16, name=f"xT{k}") for k in range(KT)]
    yall_sb = [big.tile([128, T], F16, name=f"ya{k}") for k in range(KT)]
    wqk_sb = [big.tile([128, 512], F16, name=f"wqk{k}") for k in range(KT)]
    wv_sb = [big.tile([128, 256], F16, name=f"wv{k}") for k in range(KT)]
    wproj_sb = [big.tile([128, 256], F16, name=f"wp{k}") for k in range(KT)]
    vbias_sb = big.tile([128, 256], F32, name="vbias")
    bqk_sb = [big.tile([128, 1], F32, name=f"bqk{m}") for m in range(4)]
    bpr_sb = [big.tile([128, 1], F32, name=f"bpr{m}") for m in range(2)]
    rmask_sb = [big.tile([128, T], F16, name=f"rm{m}") for m in range(2)]
    triu_sb = big.tile([128, 128], F16, name="triu")
    ones09 = big.tile([128, 64], F16, name="ones09")
    expb_sb = big.tile([128, 1], F32, name="expb")

    # ---- rotating SBUF pools ----
    mpool = ctx.enter_context(tc.tile_pool(name="mask", bufs=4))
    apool = ctx.enter_context(tc.tile_pool(name="araw", bufs=2))
    dpool = ctx.enter_context(tc.tile_pool(name="adrop", bufs=2))
    cpool = ctx.enter_context(tc.tile_pool(name="csum", bufs=2))
    rpool = ctx.enter_context(tc.tile_pool(name="recip", bufs=4))
    ypool = ctx.enter_context(tc.tile_pool(name="yj", bufs=2))
    opool = ctx.enter_context(tc.tile_pool(name="oproj", bufs=2))

    # ---- load weights and small constants (before xT so PE starts early) --
    for k in range(KT):
        nc.sync.dma_start(wqk_sb[k][:], wqk[k * 128:(k + 1) * 128, :])
    for k in range(KT):
        nc.sync.dma_start(wv_sb[k][:], wv[k * 128:(k + 1) * 128, :])
    nc.sync.dma_start(vbias_sb[:], vbias[:, :])
    for m in range(4):
        nc.sync.dma_start(bqk_sb[m][:], bqk.ap()[m])
    for m in range(2):
        nc.sync.dma_start(bpr_sb[m][:], bpr.ap()[m])
        nc.sync.dma_start(rmask_sb[m][:], rmaskT[m * 128:(m + 1) * 128, :])
    for k in range(KT):
        nc.sync.dma_start(wproj_sb[k][:], wproj[k * 128:(k + 1) * 128, :])
    nc.sync.dma_start(triu_sb[:], triu_in[:, :])
    nc.vector.memset(ones09[:], KEEP)
    nc.vector.memset(expb_sb[:], EXP_BIAS)

    # xT arrives as [C-tile, 512-token-slice] pieces, two slices up front,
    # one more per super-iteration (emitted inside qkv(n)).
    def load_x_slice(n):
        for k in range(KT):
            nc.sync.dma_start(xT_sb[k][:, n * 512:(n + 1) * 512],
                              xT[k * 128:(k + 1) * 128,
                                 n * 512:(n + 1) * 512])

    load_x_slice(0)
    load_x_slice(1)

    def qkv(n):
        """Q^T/K^T/V projection for token slice n (fp16)."""
        if n + 2 < NT512:
            load_x_slice(n + 2)
        sl = slice(n * 512, (n + 1) * 512)
        # Q^T,K^T: out[feat, tok]; m: 0=q-pair0, 1=q-pair1, 2=k-pair0, 3=k-pair1
        for m in range(4):
            ps = ps_u if m % 2 == 0 else ps_w
            for k in range(KT):
                nc.tensor.matmul(
                    ps[:],
                    wqk_sb[k][:, m * 128:(m + 1) * 128],
                    xT_sb[k][:, sl],
                    start=(k == 0), stop=(k == KT - 1))
            dest = qT_sb[m] if m < 2 else kT_sb[m - 2]
            nc.vector.tensor_scalar(dest[:, sl], ps[:], bqk_sb[m][:, 0:1],
                                    None, ALU.add)
        # V: natural layout [tok, vfeat], bias added via broadcast tile
        for t in range(4):
            q = 4 * n + t
            ps = ps_u if t % 2 == 0 else ps_w
            for k in range(KT):
                nc.tensor.matmul(
                    ps[:, 0:256],
                    xT_sb[k][:, q * 128:(q + 1) * 128],
                    wv_sb[k][:, 0:256],
                    start=(k == 0), stop=(k == KT - 1))
            nc.vector.tensor_tensor(
                v_sb[:, q * 256:(q + 1) * 256], ps[:, 0:256], vbias_sb[:],
                ALU.add)

    def attn(j):
        """Causal attention for query slice j (both head pairs), then
        normalize and ship y^T into the chunked AllGather input."""
        for p in range(PAIRS):
            n_i = 4 * j + 4  # tk tiles needed (block-causal)
            csum = cpool.tile([128, 1024], F16, tag="csum")
            for i in range(n_i):
                r = max(0, i - 4 * j)
                off = 128 * r
                ps_s = psA if i % 2 == 0 else psB
                # --- S^T = K^T.T @ Q^T (the two heads use disjoint
                # contraction partitions -> run concurrently on the PE) ---
                for h in range(2):
                    nc.tensor.matmul(
                        ps_s[:, h * 512 + off:h * 512 + 512],
                        kT_sb[p][h * 64:(h + 1) * 64,
                                 i * 128:(i + 1) * 128],
                        qT_sb[p][h * 64:(h + 1) * 64,
                                 j * 512 + off:(j + 1) * 512],
                        start=True, stop=True)
                # --- exp (valid range only) ---
                a_raw = apool.tile([128, 1024], F16, tag="araw")
                for h in range(2):
                    nc.scalar.activation(
                        a_raw[:, h * 512 + off:h * 512 + 512],
                        ps_s[:, h * 512 + off:h * 512 + 512],
                        AF.Exp, bias=expb_sb[:, 0:1], scale=1.0)
                # --- causal triangle on diagonal blocks ---
                if i >= 4 * j:
                    for h in range(2):
                        s_ = a_raw[:, h * 512 + off:h * 512 + off + 128]
                        nc.vector.tensor_tensor(s_, s_, triu_sb[:], ALU.mult)
                # --- dropout mask (fp8 {0,1}) ---
                m_tile = mpool.tile([128, 1024], F8, tag="mask")
                for h in range(2):
                    nc.sync.dma_start(
                        m_tile[:, h * 512 + off:h * 512 + 512],
                        maskT[2 * p + h, i * 128:(i + 1) * 128,
                              j * 512 + off:(j + 1) * 512])
                a_drop = dpool.tile([128, 1024], F16, tag="adrop")
                ad_v = a_drop[:].rearrange("p (h q) -> p h q", h=2)[:, :, off:512]
                ar_v = a_raw[:].rearrange("p (h q) -> p h q", h=2)[:, :, off:512]
                mt_v = m_tile[:].rearrange("p (h q) -> p h q", h=2)[:, :, off:512]
                nc.vector.tensor_tensor(ad_v, ar_v, mt_v, ALU.mult)
                # --- softmax denominator partials accumulate on DVE ---
                if i == 0:
                    nc.vector.tensor_scalar(csum[:], a_raw[:], 1.0, None,
                                            ALU.mult)
                else:
                    cs_v = csum[:].rearrange("p (h q) -> p h q", h=2)[:, :, off:512]
                    nc.vector.tensor_tensor(cs_v, cs_v, ar_v, ALU.add)
                # --- AV (head A -> ps_y rows 0:64, head B -> ps_o 64:128) ---
                for h in range(2):
                    av_bank = ps_y if h == 0 else ps_o
                    nc.tensor.matmul(
                        av_bank[64 * h:64 * h + 64, off:512],
                        v_sb[:, i * 256 + (2 * p + h) * 64:
                             i * 256 + (2 * p + h) * 64 + 64],
                        a_drop[:, h * 512 + off:h * 512 + 512],
                        start=(i == 0),
                        stop=(i == n_i - 1),
                        skip_group_check=True)
            # --- denominator: 0.9*den broadcast onto 64 partitions ---
            nc.tensor.matmul(ps_u[0:64, 0:512], ones09[:, 0:64],
                             csum[:, 0:512], start=True, stop=True,
                             skip_group_check=True)
            nc.tensor.matmul(ps_w[64:128, 0:512], ones09[:, 0:64],
                             csum[:, 512:1024], start=True, stop=True,
                             skip_group_check=True)
            denA = rpool.tile([64, 512], F32, tag="denA")
            denB = rpool.tile([64, 512], F32, tag="denB")
            nc.scalar.copy(denA[:], ps_u[0:64, :])
            nc.scalar.copy(denB[:], ps_w[64:128, :])
            recipA = rpool.tile([64, 512], F32, tag="recipA")
            recipB = rpool.tile([64, 512], F32, tag="recipB")
            nc.vector.reciprocal_approx_fast(recipA[:], denA[:])
            nc.vector.reciprocal_approx_fast(recipB[:], denB[:])
            yj = ypool.tile([128, 512], F16, tag="yj")
            nc.vector.tensor_tensor(yj[0:64, :], ps_y[0:64, :], recipA[:],
                                    ALU.mult)
            nc.vector.tensor_tensor(yj[64:128, :], ps_o[64:128, :],
                                    recipB[:], ALU.mult)
            nc.sync.dma_start(ag_in[j][128 * p:128 * p + 128, :], yj[:])
        nc.gpsimd.collective_compute(
            "AllGather", mybir.AluOpType.bypass,
            replica_groups=[[0, 1, 2, 3], [4, 5, 6, 7]],
            ins=[ag_in[j].ap()], outs=[ag_out[j].ap()])

    def gather_reads(j):
        """Pull the AllGathered y^T slice into SBUF (scalar-engine DMA
        queue: by emission time collective j is done, so no Act stall)."""
        for k in range(KT):
            nc.scalar.dma_start(yall_sb[k][:, j * 512:(j + 1) * 512],
                                ag_out[j][k * 128:(k + 1) * 128, :])

    def proj(jj):
        """Output projection for token slice jj (our 256 feature rows)."""
        sl = slice(jj * 512, (jj + 1) * 512)
        for m in range(2):
            ps = ps_u if m == 0 else ps_w
            for k in range(KT):
                nc.tensor.matmul(
                    ps[:],
                    wproj_sb[k][:, m * 128:(m + 1) * 128],
                    yall_sb[k][:, sl],
                    start=(k == 0), stop=(k == KT - 1))
            o_m = opool.tile([128, 512], F32, tag="oproj")
            nc.vector.scalar_tensor_tensor(
                o_m[:], ps[:], bpr_sb[m][:, 0:1], rmask_sb[m][:, sl],
                ALU.add, ALU.mult)
            nc.scalar.dma_start(out[m * 128:(m + 1) * 128, sl], o_m[:])

    for n in range(NT512):
        qkv(n)
        attn(n)
        if n >= 1:
            gather_reads(n - 1)
            proj(n - 1)
    gather_reads(NT512 - 1)
    proj(NT512 - 1)

    ctx.close()


def prep_inputs(x, Wqkv, bqkv, Wproj, bproj, attn_drop_mask, resid_drop_mask):
    """Shard + lay out the full inputs for the 8 cores."""
    x = np.asarray(x, np.float32)
    Wqkv = np.asarray(Wqkv, np.float32)
    bqkv = np.asarray(bqkv, np.float32)
    Wproj = np.asarray(Wproj, np.float32)
    bproj = np.asarray(bproj, np.float32)
    attn_drop_mask = np.asarray(attn_drop_mask, bool)
    resid_drop_mask = np.asarray(resid_drop_mask, bool)

    f8 = ml_dtypes.float8_e4m3
    tril = np.tril(np.ones((T, T), dtype=bool))
    triu128 = np.triu(np.ones((128, 128), np.float16))
    qscale = np.float32(1.0 / np.sqrt(HD))
    in_maps = []
    for core in range(N_CORES):
        b, g = divmod(core, GROUPS)
        cs = slice(g * 256, (g + 1) * 256)  # this group's feature rows
        wq_c = Wqkv[:, 0:1024][:, cs] * qscale  # fold 1/sqrt(hd) into q
        wk_c = Wqkv[:, 1024:2048][:, cs]
        wqk_c = np.concatenate([wq_c, wk_c], axis=1).astype(np.float16)
        wv_c = np.ascontiguousarray(Wqkv[:, 2048:3072][:, cs]).astype(np.float16)
        bq = (bqkv[0:1024][cs] * qscale).astype(np.float32)
        bk = bqkv[1024:2048][cs]
        bv = bqkv[2048:3072][cs]
        bqk_c = np.stack([bq[0:128], bq[128:256], bk[0:128], bk[128:256]])
        bqk_c = bqk_c.reshape(4, 128, 1).astype(np.float32)
        vbias_c = np.broadcast_to(bv, (128, 256)).astype(np.float32).copy()
        # combined causal & dropout mask, transposed to [tk, tq], fp8 {0,1}
        m = attn_drop_mask[b, g * HPG:(g + 1) * HPG] & tril
        maskT_c = np.ascontiguousarray(m.transpose(0, 2, 1)).astype(f8)
        rmaskT_c = np.ascontiguousarray(
            resid_drop_mask[b, :, cs].T).astype(np.float16)
        # fold the residual-dropout 1/KEEP into Wproj and bproj
        wproj_c = (Wproj[:, cs] / np.float32(KEEP)).astype(np.float16)
        bpr_c = (bproj[cs] / np.float32(KEEP)).astype(np.float32)
        bpr_c = bpr_c.reshape(2, 128, 1)
        in_maps.append(dict(
            xT=np.ascontiguousarray(x[b].T).astype(np.float16),
            wqk=np.ascontiguousarray(wqk_c),
            wv=wv_c,
            vbias=vbias_c,
            bqk=bqk_c,
            wproj=np.ascontiguousarray(wproj_c),
            bpr=bpr_c,
            maskT=maskT_c,
            rmaskT=rmaskT_c,
            triu=triu128,
        ))
    return in_maps


_NC_CACHE = {}


def _get_nc():
    if "nc" not in _NC_CACHE:
        _NC_CACHE["nc"] = build_kernel()
    return _NC_CACHE["nc"]


def kernel(trace=False, **inputs):
    nc = _get_nc()
    in_maps = prep_inputs(**inputs)
    res = run_bass_kernel_spmd(nc, in_maps, core_ids=list(range(N_CORES)),
                               trace=trace)
    y = np.empty((B, T, C), np.float32)
    for core in range(N_CORES):
        b, g = divmod(core, GROUPS)
        y[b, :, g * 256:(g + 1) * 256] = res.results[core]["out"].T
    kernel.last_result = res
    return y


# revision 6
# speedup vs baseline: 1.9331x; 1.1335x over previous
"""Trainium2 Bass kernel for CausalSelfAttention (B=2, T=2048, C=1024, H=16).

Sharding: 8 cores = 2 batches x 4 head-groups (4 heads each).
Per core, interleaved by 512-token slice n:
  QKV proj (fp16) for slice n -> causal attention for query slice j=n
  (S^T layout, fp16, S issued one block ahead of AV; exp on Act;
  dropout-mask mult + softmax-denominator accumulation on DVE) ->
  per-head-pair fp16 AllGather of the y^T slice across the 4 cores of
  the batch -> output proj for the previous slice (256 feature rows
  each), one chunk behind so the collectives stay hidden.
"""

import sys

sys.path.insert(0, "/opt/trn_rl_repo")

import numpy as np

import concourse.bass as bass
import concourse.mybir as mybir
import concourse.tile as tile
from concourse import bacc
from concourse.bass_utils import run_bass_kernel_spmd

F32 = mybir.dt.float32
F16 = mybir.dt.float16

B, T, C, H = 2, 2048, 1024, 16
HD = C // H  # 64
N_CORES = 8
GROUPS = 4            # head groups (one per core within a batch)
HPG = H // GROUPS     # heads per group = 4
PAIRS = HPG // 2      # head pairs per core = 2
KEEP = 0.9
EXP_BIAS = -3.0       # exp(s - 3): cancels in normalization, avoids overflow

NT512 = T // 512      # 4 token slices of 512
KT = C // 128         # 8 contraction tiles


def build_kernel():
    nc = bacc.Bacc("TRN2", target_bir_lowering=False, debug=False,
                   num_devices=N_CORES)

    # ---- per-core DRAM I/O ----
    xT = nc.dram_tensor("xT", [C, T], F16, kind="ExternalInput")
    wqk = nc.dram_tensor("wqk", [C, 512], F16, kind="ExternalInput")
    wv = nc.dram_tensor("wv", [C, 256], F16, kind="ExternalInput")
    vbias = nc.dram_tensor("vbias", [128, 256], F32, kind="ExternalInput")
    bqk = nc.dram_tensor("bqk", [4, 128, 1], F32, kind="ExternalInput")
    wproj = nc.dram_tensor("wproj", [C, 256], F16, kind="ExternalInput")
    bpr = nc.dram_tensor("bpr", [2, 128, 1], F32, kind="ExternalInput")
    maskT = nc.dram_tensor("maskT", [HPG, T, T], F16, kind="ExternalInput")
    rmaskT = nc.dram_tensor("rmaskT", [256, T], F16, kind="ExternalInput")
    triu_in = nc.dram_tensor("triu", [128, 128], F16, kind="ExternalInput")
    out = nc.dram_tensor("out", [256, T], F32, kind="ExternalOutput")

    # internal DRAM for the chunked AllGather (per head-pair, per slice)
    ag_in = [[nc.dram_tensor(f"ag_in{p}_{j}", [128, 512], F16)
              for j in range(NT512)] for p in range(PAIRS)]
    ag_out = [[nc.dram_tensor(f"ag_out{p}_{j}", [512, 512], F16)
               for j in range(NT512)] for p in range(PAIRS)]

    with tile.TileContext(nc) as tc:
        _build_body(nc, tc, xT, wqk, wv, vbias, bqk, wproj, bpr, maskT,
                    rmaskT, triu_in, out, ag_in, ag_out)
    nc.compile()
    return nc


def _build_body(nc, tc, xT, wqk, wv, vbias, bqk, wproj, bpr, maskT,
                rmaskT, triu_in, out, ag_in, ag_out):
    from contextlib import ExitStack
    ctx = ExitStack()
    AF = mybir.ActivationFunctionType
    ALU = mybir.AluOpType

    # ---- PSUM (8 banks) ----
    psA = ctx.enter_context(nc.psum_tensor([128, 1024], F32))   # S ping + denA
    psB = ctx.enter_context(nc.psum_tensor([128, 1024], F32))   # S pong + denB
    ps_y = ctx.enter_context(nc.psum_tensor([128, 512], F32))   # AV head A
    ps_o = ctx.enter_context(nc.psum_tensor([128, 512], F32))   # AV head B
    ps_u = ctx.enter_context(nc.psum_tensor([128, 512], F32))   # qkv/proj ping
    ps_w = ctx.enter_context(nc.psum_tensor([128, 512], F32))   # qkv/proj pong

    # ---- persistent SBUF ----
    big = ctx.enter_context(tc.tile_pool(name="big", bufs=1))
    qT_sb = [big.tile([128, T], F16, name=f"qT{p}") for p in range(PAIRS)]
    kT_sb = [big.tile([128, T], F16, name=f"kT{p}") for p in range(PAIRS)]
    v_sb = big.tile([128, (T // 128) * 256], F16, name="v")
    xT_sb = [big.tile([128, T], F16, name=f"xT{k}") for k in range(KT)]
    yall_sb = [big.tile([128, T], F16, name=f"ya{k}") for k in range(KT)]
    wqk_sb = [big.tile([128, 512], F16, name=f"wqk{k}") for k in range(KT)]
    wv_sb = [big.tile([128, 256], F16, name=f"wv{k}") for k in range(KT)]
    wproj_sb = [big.tile([128, 256], F16, name=f"wp{k}") for k in range(KT)]
    vbias_sb = big.tile([128, 256], F32, name="vbias")
    bqk_sb = [big.tile([128, 1], F32, name=f"bqk{m}") for m in range(4)]
    bpr_sb = [big.tile([128, 1], F32, name=f"bpr{m}") for m in range(2)]
    rmask_sb = [big.tile([128, T], F16, name=f"rm{m}") for m in range(2)]
    triu_sb = big.tile([128, 128], F16, name="triu")
    ones09 = big.tile([128, 128], F16, name="ones09")
    expb_sb = big.tile([128, 1], F32, name="expb")

    # ---- rotating SBUF pools ----
    mpool = ctx.enter_context(tc.tile_pool(name="mask", bufs=4))
    apool = ctx.enter_context(tc.tile_pool(name="araw", bufs=3))
    dpool = ctx.enter_context(tc.tile_pool(name="adrop", bufs=3))
    cpool = ctx.enter_context(tc.tile_pool(name="csum", bufs=2))
    rpool = ctx.enter_context(tc.tile_pool(name="recip", bufs=4))
    ypool = ctx.enter_context(tc.tile_pool(name="yj", bufs=2))
    opool = ctx.enter_context(tc.tile_pool(name="oproj", bufs=2))

    def load_x_slice(n):
        for k in range(KT):
            nc.sync.dma_start(xT_sb[k][:, n * 512:(n + 1) * 512],
                              xT[k * 128:(k + 1) * 128,
                                 n * 512:(n + 1) * 512])

    # ---- preamble loads: what qkv(0) needs first, the bulk deferred ----
    for k in range(KT):
        nc.sync.dma_start(wqk_sb[k][:], wqk[k * 128:(k + 1) * 128, :])
    load_x_slice(0)
    for k in range(KT):
        nc.sync.dma_start(wv_sb[k][:], wv[k * 128:(k + 1) * 128, :])
    for m in range(4):
        nc.sync.dma_start(bqk_sb[m][:], bqk.ap()[m])
    nc.sync.dma_start(vbias_sb[:], vbias[:, :])
    load_x_slice(1)
    nc.sync.dma_start(triu_sb[:], triu_in[:, :])
    for m in range(2):
        nc.sync.dma_start(bpr_sb[m][:], bpr.ap()[m])
        nc.sync.dma_start(rmask_sb[m][:], rmaskT[m * 128:(m + 1) * 128, :])
    for k in range(KT):
        nc.sync.dma_start(wproj_sb[k][:], wproj[k * 128:(k + 1) * 128, :])
    nc.vector.memset(ones09[:], KEEP)
    nc.vector.memset(expb_sb[:], EXP_BIAS)

    def qkv(n):
        """Q^T/K^T/V projection for token slice n (fp16)."""
        sl = slice(n * 512, (n + 1) * 512)
        # Q^T,K^T: out[feat, tok]; m: 0=q-pair0, 1=q-pair1, 2=k-pair0, 3=k-pair1
        for m in range(4):
            ps = ps_u if m % 2 == 0 else ps_w
            for k in range(KT):
                nc.tensor.matmul(
                    ps[:],
                    wqk_sb[k][:, m * 128:(m + 1) * 128],
                    xT_sb[k][:, sl],
                    start=(k == 0), stop=(k == KT - 1))
            dest = qT_sb[m] if m < 2 else kT_sb[m - 2]
            nc.scalar.add(dest[:, sl], ps[:], bqk_sb[m][:, 0:1])
        # V: natural layout [tok, vfeat], bias added via broadcast tile
        for t in range(4):
            q = 4 * n + t
            ps = ps_u if t % 2 == 0 else ps_w
            for k in range(KT):
                nc.tensor.matmul(
                    ps[:, 0:256],
                    xT_sb[k][:, q * 128:(q + 1) * 128],
                    wv_sb[k][:, 0:256],
                    start=(k == 0), stop=(k == KT - 1))
            nc.vector.tensor_tensor(
                v_sb[:, q * 256:(q + 1) * 256], ps[:, 0:256], vbias_sb[:],
                ALU.add)

    def attn(j):
        """Causal attention for query slice j (both head pairs). The PE
        stream is software-pipelined: S for block i+1 issues before AV for
        block i, so the exp/mask work on Act/DVE is off the PE critical
        path. y^T ships per head pair into its own small AllGather."""
        for p in range(PAIRS):
            n_i = 4 * j + 4  # tk tiles needed (block-causal)
            csum = cpool.tile([128, 1024], F16, tag="csum")

            def s_block(i):
                r = max(0, i - 4 * j)
                off = 128 * r
                ps_s = psA if i % 2 == 0 else psB
                for h in range(2):
                    nc.tensor.matmul(
                        ps_s[:, h * 512 + off:h * 512 + 512],
                        kT_sb[p][h * 64:(h + 1) * 64,
                                 i * 128:(i + 1) * 128],
                        qT_sb[p][h * 64:(h + 1) * 64,
                                 j * 512 + off:(j + 1) * 512],
                        start=True, stop=True)
                # exp (Act) into a_raw, valid range only
                a_raw = apool.tile([128, 1024], F16, tag="araw")
                for h in range(2):
                    nc.scalar.activation(
                        a_raw[:, h * 512 + off:h * 512 + 512],
                        ps_s[:, h * 512 + off:h * 512 + 512],
                        AF.Exp, bias=expb_sb[:, 0:1], scale=1.0)
                if i >= 4 * j:  # causal triangle on diagonal blocks
                    for h in range(2):
                        s_ = a_raw[:, h * 512 + off:h * 512 + off + 128]
                        nc.vector.tensor_tensor(s_, s_, triu_sb[:], ALU.mult)
                # dropout mask: both head planes in one 3D DMA
                m_tile = mpool.tile([128, 1024], F16, tag="mask")
                mt_v = m_tile[:].rearrange("p (h q) -> p h q", h=2)[:, :, off:512]
                nc.sync.dma_start(
                    mt_v,
                    maskT.ap()[2 * p:2 * p + 2, i * 128:(i + 1) * 128,
                               j * 512 + off:(j + 1) * 512]
                    .rearrange("h p q -> p h q"))
                a_drop = dpool.tile([128, 1024], F16, tag="adrop")
                ad_v = a_drop[:].rearrange("p (h q) -> p h q", h=2)[:, :, off:512]
                ar_v = a_raw[:].rearrange("p (h q) -> p h q", h=2)[:, :, off:512]
                nc.vector.tensor_tensor(ad_v, ar_v, mt_v, ALU.mult)
                # softmax denominator partials accumulate on DVE
                if i == 0:
                    nc.vector.tensor_scalar(csum[:], a_raw[:], 1.0, None,
                                            ALU.mult)
                else:
                    cs_v = csum[:].rearrange("p (h q) -> p h q", h=2)[:, :, off:512]
                    nc.vector.tensor_tensor(cs_v, cs_v, ar_v, ALU.add)
                return off, a_drop

            def av_block(i, off, a_drop):
                for h in range(2):
                    av_bank = ps_y if h == 0 else ps_o
                    nc.tensor.matmul(
                        av_bank[64 * h:64 * h + 64, off:512],
                        v_sb[:, i * 256 + (2 * p + h) * 64:
                             i * 256 + (2 * p + h) * 64 + 64],
                        a_drop[:, h * 512 + off:h * 512 + 512],
                        start=(i == 0),
                        stop=(i == n_i - 1),
                        skip_group_check=True)

            stage = []  # software pipeline: S for i+1 issues before AV for i
            for i in range(n_i):
                stage.append((i,) + s_block(i))
                if len(stage) > 1:
                    av_block(*stage.pop(0))
            av_block(*stage.pop(0))
            # --- denominator: 0.9*den, the two heads on concurrent column
            # groups of the PE, landing in the (now free) S banks ---
            nc.tensor.matmul(psA[0:64, 0:512], ones09[:, 0:64],
                             csum[:, 0:512], start=True, stop=True,
                             skip_group_check=True)
            nc.tensor.matmul(psB[64:128, 0:512], ones09[:, 64:128],
                             csum[:, 512:1024], start=True, stop=True,
                             skip_group_check=True)
            denA = rpool.tile([64, 512], F32, tag="denA")
            denB = rpool.tile([64, 512], F32, tag="denB")
            nc.scalar.copy(denA[:], psA[0:64, 0:512])
            nc.scalar.copy(denB[:], psB[64:128, 0:512])
            recipA = rpool.tile([64, 512], F32, tag="recipA")
            recipB = rpool.tile([64, 512], F32, tag="recipB")
            nc.vector.reciprocal_approx_fast(recipA[:], denA[:])
            nc.vector.reciprocal_approx_fast(recipB[:], denB[:])
            yj = ypool.tile([128, 512], F16, tag="yj")
            nc.vector.tensor_tensor(yj[0:64, :], ps_y[0:64, :], recipA[:],
                                    ALU.mult)
            nc.vector.tensor_tensor(yj[64:128, :], ps_o[64:128, :],
                                    recipB[:], ALU.mult)
            nc.sync.dma_start(ag_in[p][j][:, :], yj[:])
            nc.gpsimd.collective_compute(
                "AllGather", mybir.AluOpType.bypass,
                replica_groups=[[0, 1, 2, 3], [4, 5, 6, 7]],
                ins=[ag_in[p][j].ap()], outs=[ag_out[p][j].ap()])
        # stream the slice-after-next of x while attention runs
        if j + 2 < NT512:
            load_x_slice(j + 2)

    def gather_reads(j):
        """Pull the AllGathered y^T slice into SBUF (scalar-engine DMA
        queue; by emission time the collectives for slice j are done)."""
        for p in range(PAIRS):
            for g in range(GROUPS):
                nc.scalar.dma_start(
                    yall_sb[2 * g + p][:, j * 512:(j + 1) * 512],
                    ag_out[p][j][g * 128:(g + 1) * 128, :])

    # proj contraction order: pair-0 feature tiles first so the pair-1
    # AllGather of the final slice can still be in flight
    K_ORDER = ([2 * g for g in range(GROUPS)]
               + [2 * g + 1 for g in range(GROUPS)])

    def proj(jj):
        """Output projection for token slice jj (our 256 feature rows)."""
        sl = slice(jj * 512, (jj + 1) * 512)
        for m in range(2):
            ps = ps_u if m == 0 else ps_w
            for ki, k in enumerate(K_ORDER):
                nc.tensor.matmul(
                    ps[:],
                    wproj_sb[k][:, m * 128:(m + 1) * 128],
                    yall_sb[k][:, sl],
                    start=(ki == 0), stop=(ki == KT - 1))
            o_m = opool.tile([128, 512], F32, tag="oproj")
            nc.vector.scalar_tensor_tensor(
                o_m[:], ps[:], bpr_sb[m][:, 0:1], rmask_sb[m][:, sl],
                ALU.add, ALU.mult)
            nc.scalar.dma_start(out[m * 128:(m + 1) * 128, sl], o_m[:])

    for n in range(NT512):
        qkv(n)
        attn(n)
        if n >= 1:
            gather_reads(n - 1)
            proj(n - 1)
    gather_reads(NT512 - 1)
    proj(NT512 - 1)

    ctx.close()


def prep_inputs(x, Wqkv, bqkv, Wproj, bproj, attn_drop_mask, resid_drop_mask):
    """Shard + lay out the full inputs for the 8 cores."""
    x = np.asarray(x, np.float32)
    Wqkv = np.asarray(Wqkv, np.float32)
    bqkv = np.asarray(bqkv, np.float32)
    Wproj = np.asarray(Wproj, np.float32)
    bproj = np.asarray(bproj, np.float32)
    attn_drop_mask = np.asarray(attn_drop_mask, bool)
    resid_drop_mask = np.asarray(resid_drop_mask, bool)

    tril = np.tril(np.ones((T, T), dtype=bool))
    triu128 = np.triu(np.ones((128, 128), np.float16))
    qscale = np.float32(1.0 / np.sqrt(HD))
    in_maps = []
    for core in range(N_CORES):
        b, g = divmod(core, GROUPS)
        cs = slice(g * 256, (g + 1) * 256)  # this group's feature rows
        wq_c = Wqkv[:, 0:1024][:, cs] * qscale  # fold 1/sqrt(hd) into q
        wk_c = Wqkv[:, 1024:2048][:, cs]
        wqk_c = np.concatenate([wq_c, wk_c], axis=1).astype(np.float16)
        wv_c = np.ascontiguousarray(Wqkv[:, 2048:3072][:, cs]).astype(np.float16)
        bq = (bqkv[0:1024][cs] * qscale).astype(np.float32)
        bk = bqkv[1024:2048][cs]
        bv = bqkv[2048:3072][cs]
        bqk_c = np.stack([bq[0:128], bq[128:256], bk[0:128], bk[128:256]])
        bqk_c = bqk_c.reshape(4, 128, 1).astype(np.float32)
        vbias_c = np.broadcast_to(bv, (128, 256)).astype(np.float32).copy()
        # combined causal & dropout mask, transposed to [tk, tq], {0,1}
        m = attn_drop_mask[b, g * HPG:(g + 1) * HPG] & tril
        maskT_c = np.ascontiguousarray(
            m.transpose(0, 2, 1)).astype(np.float16)
        rmaskT_c = np.ascontiguousarray(
            resid_drop_mask[b, :, cs].T).astype(np.float16)
        # fold the residual-dropout 1/KEEP into Wproj and bproj
        wproj_c = (Wproj[:, cs] / np.float32(KEEP)).astype(np.float16)
        bpr_c = (bproj[cs] / np.float32(KEEP)).astype(np.float32)
        bpr_c = bpr_c.reshape(2, 128, 1)
        in_maps.append(dict(
            xT=np.ascontiguousarray(x[b].T).astype(np.float16),
            wqk=np.ascontiguousarray(wqk_c),
            wv=wv_c,
            vbias=vbias_c,
            bqk=bqk_c,
            wproj=np.ascontiguousarray(wproj_c),
            bpr=bpr_c,
            maskT=maskT_c,
            rmaskT=rmaskT_c,
            triu=triu128,
        ))
    return in_maps


_NC_CACHE = {}


def _get_nc():
    if "nc" not in _NC_CACHE:
        _NC_CACHE["nc"] = build_kernel()
    return _NC_CACHE["nc"]


def kernel(trace=False, **inputs):
    nc = _get_nc()
    in_maps = prep_inputs(**inputs)
    res = run_bass_kernel_spmd(nc, in_maps, core_ids=list(range(N_CORES)),
                               trace=trace)
    y = np.empty((B, T, C), np.float32)
    for core in range(N_CORES):
        b, g = divmod(core, GROUPS)
        y[b, :, g * 256:(g + 1) * 256] = res.results[core]["out"].T
    kernel.last_result = res
    return y


# revision 7
# speedup vs baseline: 2.0642x; 1.0678x over previous
"""Trainium2 Bass kernel for CausalSelfAttention (B=2, T=2048, C=1024, H=16).

Sharding: 8 cores = 2 batches x 4 head-groups (4 heads each).
Per core, interleaved by 512-token slice n:
  QKV proj (fp16) for slice n -> causal attention for query slice j=n
  (S^T layout, fp16, S issued one block ahead of AV; exp on Act;
  dropout-mask mult + softmax-denominator accumulation on DVE) ->
  per-head-pair fp16 AllGather of the y^T slice across the 4 cores of
  the batch -> output proj for the previous slice (256 feature rows
  each), one chunk behind so the collectives stay hidden.
"""

import sys

sys.path.insert(0, "/opt/trn_rl_repo")

import numpy as np

import concourse.bass as bass
import concourse.mybir as mybir
import concourse.tile as tile
from concourse import bacc
from concourse.bass_utils import run_bass_kernel_spmd

F32 = mybir.dt.float32
F16 = mybir.dt.float16

B, T, C, H = 2, 2048, 1024, 16
HD = C // H  # 64
N_CORES = 8
GROUPS = 4            # head groups (one per core within a batch)
HPG = H // GROUPS     # heads per group = 4
PAIRS = HPG // 2      # head pairs per core = 2
KEEP = 0.9
EXP_BIAS = -3.0       # exp(s - 3): cancels in normalization, avoids overflow

NT512 = T // 512      # 4 token slices of 512
KT = C // 128         # 8 contraction tiles


def build_kernel():
    nc = bacc.Bacc("TRN2", target_bir_lowering=False, debug=False,
                   num_devices=N_CORES)

    # ---- per-core DRAM I/O ----
    xT = nc.dram_tensor("xT", [C, T], F16, kind="ExternalInput")
    wqk = nc.dram_tensor("wqk", [C, 512], F16, kind="ExternalInput")
    wv = nc.dram_tensor("wv", [C, 256], F16, kind="ExternalInput")
    vbias = nc.dram_tensor("vbias", [128, 256], F32, kind="ExternalInput")
    bqk = nc.dram_tensor("bqk", [4, 128, 1], F32, kind="ExternalInput")
    wproj = nc.dram_tensor("wproj", [C, 256], F16, kind="ExternalInput")
    bpr = nc.dram_tensor("bpr", [2, 128, 1], F32, kind="ExternalInput")
    maskT = nc.dram_tensor("maskT", [HPG, T, T], F16, kind="ExternalInput")
    rmaskT = nc.dram_tensor("rmaskT", [256, T], F16, kind="ExternalInput")
    triu_in = nc.dram_tensor("triu", [128, 128], F16, kind="ExternalInput")
    out = nc.dram_tensor("out", [256, T], F32, kind="ExternalOutput")

    # internal DRAM for the chunked AllGather (per head-pair, per slice)
    ag_in = [[nc.dram_tensor(f"ag_in{p}_{j}", [128, 512], F16)
              for j in range(NT512)] for p in range(PAIRS)]
    ag_out = [[nc.dram_tensor(f"ag_out{p}_{j}", [512, 512], F16)
               for j in range(NT512)] for p in range(PAIRS)]

    with tile.TileContext(nc) as tc:
        _build_body(nc, tc, xT, wqk, wv, vbias, bqk, wproj, bpr, maskT,
                    rmaskT, triu_in, out, ag_in, ag_out)
    nc.compile()
    return nc


def _build_body(nc, tc, xT, wqk, wv, vbias, bqk, wproj, bpr, maskT,
                rmaskT, triu_in, out, ag_in, ag_out):
    from contextlib import ExitStack
    ctx = ExitStack()
    AF = mybir.ActivationFunctionType
    ALU = mybir.AluOpType

    # ---- PSUM (8 banks) ----
    psA = ctx.enter_context(nc.psum_tensor([128, 1024], F32))   # S ping + denA
    psB = ctx.enter_context(nc.psum_tensor([128, 1024], F32))   # S pong + denB
    ps_y = ctx.enter_context(nc.psum_tensor([128, 512], F32))   # AV head A
    ps_o = ctx.enter_context(nc.psum_tensor([128, 512], F32))   # AV head B
    ps_u = ctx.enter_context(nc.psum_tensor([128, 512], F32))   # qkv/proj ping
    ps_w = ctx.enter_context(nc.psum_tensor([128, 512], F32))   # qkv/proj pong

    # ---- persistent SBUF ----
    big = ctx.enter_context(tc.tile_pool(name="big", bufs=1))
    qT_sb = [big.tile([128, T], F16, name=f"qT{p}") for p in range(PAIRS)]
    kT_sb = [big.tile([128, T], F16, name=f"kT{p}") for p in range(PAIRS)]
    v_sb = big.tile([128, (T // 128) * 256], F16, name="v")
    xT_sb = [big.tile([128, T], F16, name=f"xT{k}") for k in range(KT)]
    yall_sb = [big.tile([128, T], F16, name=f"ya{k}") for k in range(KT)]
    wqk_sb = [big.tile([128, 512], F16, name=f"wqk{k}") for k in range(KT)]
    wv_sb = [big.tile([128, 256], F16, name=f"wv{k}") for k in range(KT)]
    wproj_sb = [big.tile([128, 256], F16, name=f"wp{k}") for k in range(KT)]
    vbias_sb = big.tile([128, 256], F32, name="vbias")
    bqk_sb = [big.tile([128, 1], F32, name=f"bqk{m}") for m in range(4)]
    bpr_sb = [big.tile([128, 1], F32, name=f"bpr{m}") for m in range(2)]
    rmask_sb = [big.tile([128, T], F16, name=f"rm{m}") for m in range(2)]
    triu_sb = big.tile([128, 128], F16, name="triu")
    ones09 = big.tile([128, 128], F16, name="ones09")
    expb_sb = big.tile([128, 1], F32, name="expb")

    # ---- rotating SBUF pools ----
    mpool = ctx.enter_context(tc.tile_pool(name="mask", bufs=6))
    apool = ctx.enter_context(tc.tile_pool(name="araw", bufs=3))
    dpool = ctx.enter_context(tc.tile_pool(name="adrop", bufs=3))
    cpool = ctx.enter_context(tc.tile_pool(name="csum", bufs=2))
    rpool = ctx.enter_context(tc.tile_pool(name="recip", bufs=4))
    ypool = ctx.enter_context(tc.tile_pool(name="yj", bufs=2))
    opool = ctx.enter_context(tc.tile_pool(name="oproj", bufs=2))

    def load_x_slice(n):
        for k in range(KT):
            nc.sync.dma_start(xT_sb[k][:, n * 512:(n + 1) * 512],
                              xT[k * 128:(k + 1) * 128,
                                 n * 512:(n + 1) * 512])

    # ---- preamble loads: what qkv(0) needs first, the bulk deferred ----
    for k in range(KT):
        nc.sync.dma_start(wqk_sb[k][:], wqk[k * 128:(k + 1) * 128, :])
    load_x_slice(0)
    for k in range(KT):
        nc.sync.dma_start(wv_sb[k][:], wv[k * 128:(k + 1) * 128, :])
    for m in range(4):
        nc.sync.dma_start(bqk_sb[m][:], bqk.ap()[m])
    nc.sync.dma_start(vbias_sb[:], vbias[:, :])
    load_x_slice(1)
    nc.sync.dma_start(triu_sb[:], triu_in[:, :])
    for m in range(2):
        nc.sync.dma_start(bpr_sb[m][:], bpr.ap()[m])
        nc.sync.dma_start(rmask_sb[m][:], rmaskT[m * 128:(m + 1) * 128, :])
    for k in range(KT):
        nc.sync.dma_start(wproj_sb[k][:], wproj[k * 128:(k + 1) * 128, :])
    nc.vector.memset(ones09[:], KEEP)
    nc.vector.memset(expb_sb[:], EXP_BIAS)

    def qkv(n):
        """Q^T/K^T/V projection for token slice n (fp16)."""
        sl = slice(n * 512, (n + 1) * 512)
        # Q^T,K^T: out[feat, tok]; m: 0=q-pair0, 1=q-pair1, 2=k-pair0, 3=k-pair1
        for m in range(4):
            ps = ps_u if m % 2 == 0 else ps_w
            for k in range(KT):
                nc.tensor.matmul(
                    ps[:],
                    wqk_sb[k][:, m * 128:(m + 1) * 128],
                    xT_sb[k][:, sl],
                    start=(k == 0), stop=(k == KT - 1))
            dest = qT_sb[m] if m < 2 else kT_sb[m - 2]
            nc.scalar.add(dest[:, sl], ps[:], bqk_sb[m][:, 0:1])
        # V: natural layout [tok, vfeat], bias added via broadcast tile
        for t in range(4):
            q = 4 * n + t
            ps = ps_u if t % 2 == 0 else ps_w
            for k in range(KT):
                nc.tensor.matmul(
                    ps[:, 0:256],
                    xT_sb[k][:, q * 128:(q + 1) * 128],
                    wv_sb[k][:, 0:256],
                    start=(k == 0), stop=(k == KT - 1))
            nc.vector.tensor_tensor(
                v_sb[:, q * 256:(q + 1) * 256], ps[:, 0:256], vbias_sb[:],
                ALU.add)

    def attn(j):
        """Causal attention for query slice j (both head pairs). The PE
        stream is software-pipelined: S for block i+1 issues before AV for
        block i, so the exp/mask work on Act/DVE is off the PE critical
        path. y^T ships per head pair into its own small AllGather."""
        for p in range(PAIRS):
            n_i = 4 * j + 4  # tk tiles needed (block-causal)
            csum = cpool.tile([128, 1024], F16, tag="csum")

            def s_block(i):
                r = max(0, i - 4 * j)
                off = 128 * r
                ps_s = psA if i % 2 == 0 else psB
                for h in range(2):
                    nc.tensor.matmul(
                        ps_s[:, h * 512 + off:h * 512 + 512],
                        kT_sb[p][h * 64:(h + 1) * 64,
                                 i * 128:(i + 1) * 128],
                        qT_sb[p][h * 64:(h + 1) * 64,
                                 j * 512 + off:(j + 1) * 512],
                        start=True, stop=True)
                # exp (Act) into a_raw, valid range only
                a_raw = apool.tile([128, 1024], F16, tag="araw")
                for h in range(2):
                    nc.scalar.activation(
                        a_raw[:, h * 512 + off:h * 512 + 512],
                        ps_s[:, h * 512 + off:h * 512 + 512],
                        AF.Exp, bias=expb_sb[:, 0:1], scale=1.0)
                if i >= 4 * j:  # causal triangle on diagonal blocks
                    for h in range(2):
                        s_ = a_raw[:, h * 512 + off:h * 512 + off + 128]
                        nc.vector.tensor_tensor(s_, s_, triu_sb[:], ALU.mult)
                # dropout mask: both head planes in one 3D DMA
                m_tile = mpool.tile([128, 1024], F16, tag="mask")
                mt_v = m_tile[:].rearrange("p (h q) -> p h q", h=2)[:, :, off:512]
                nc.sync.dma_start(
                    mt_v,
                    maskT.ap()[2 * p:2 * p + 2, i * 128:(i + 1) * 128,
                               j * 512 + off:(j + 1) * 512]
                    .rearrange("h p q -> p h q"))
                a_drop = dpool.tile([128, 1024], F16, tag="adrop")
                for h in range(2):
                    hs = slice(h * 512 + off, h * 512 + 512)
                    nc.vector.tensor_tensor(a_drop[:, hs], a_raw[:, hs],
                                            m_tile[:, hs], ALU.mult)
                # softmax denominator partials accumulate on DVE
                if i == 0:
                    nc.vector.tensor_scalar(csum[:], a_raw[:], 1.0, None,
                                            ALU.mult)
                else:
                    for h in range(2):
                        hs = slice(h * 512 + off, h * 512 + 512)
                        nc.vector.tensor_tensor(csum[:, hs], csum[:, hs],
                                                a_raw[:, hs], ALU.add)
                return off, a_drop

            def av_block(i, off, a_drop):
                for h in range(2):
                    av_bank = ps_y if h == 0 else ps_o
                    nc.tensor.matmul(
                        av_bank[64 * h:64 * h + 64, off:512],
                        v_sb[:, i * 256 + (2 * p + h) * 64:
                             i * 256 + (2 * p + h) * 64 + 64],
                        a_drop[:, h * 512 + off:h * 512 + 512],
                        start=(i == 0),
                        stop=(i == n_i - 1),
                        skip_group_check=True)

            stage = []  # software pipeline: S for i+1 issues before AV for i
            for i in range(n_i):
                stage.append((i,) + s_block(i))
                if len(stage) > 1:
                    av_block(*stage.pop(0))
            av_block(*stage.pop(0))
            # --- denominator: 0.9*den, the two heads on concurrent column
            # groups of the PE, landing in the (now free) S banks ---
            nc.tensor.matmul(psA[0:64, 0:512], ones09[:, 0:64],
                             csum[:, 0:512], start=True, stop=True,
                             skip_group_check=True)
            nc.tensor.matmul(psB[64:128, 0:512], ones09[:, 64:128],
                             csum[:, 512:1024], start=True, stop=True,
                             skip_group_check=True)
            denA = rpool.tile([64, 512], F32, tag="denA")
            denB = rpool.tile([64, 512], F32, tag="denB")
            nc.scalar.copy(denA[:], psA[0:64, 0:512])
            nc.scalar.copy(denB[:], psB[64:128, 0:512])
            recipA = rpool.tile([64, 512], F32, tag="recipA")
            recipB = rpool.tile([64, 512], F32, tag="recipB")
            nc.vector.reciprocal_approx_fast(recipA[:], denA[:])
            nc.vector.reciprocal_approx_fast(recipB[:], denB[:])
            yj = ypool.tile([128, 512], F16, tag="yj")
            nc.vector.tensor_tensor(yj[0:64, :], ps_y[0:64, :], recipA[:],
                                    ALU.mult)
            nc.vector.tensor_tensor(yj[64:128, :], ps_o[64:128, :],
                                    recipB[:], ALU.mult)
            nc.sync.dma_start(ag_in[p][j][:, :], yj[:])
            nc.gpsimd.collective_compute(
                "AllGather", mybir.AluOpType.bypass,
                replica_groups=[[0, 1, 2, 3], [4, 5, 6, 7]],
                ins=[ag_in[p][j].ap()], outs=[ag_out[p][j].ap()])
        # stream the slice-after-next of x while attention runs
        if j + 2 < NT512:
            load_x_slice(j + 2)

    def gather_reads(j):
        """Pull the AllGathered y^T slice into SBUF (scalar-engine DMA
        queue; by emission time the collectives for slice j are done)."""
        for p in range(PAIRS):
            for g in range(GROUPS):
                nc.scalar.dma_start(
                    yall_sb[2 * g + p][:, j * 512:(j + 1) * 512],
                    ag_out[p][j][g * 128:(g + 1) * 128, :])

    # proj contraction order: pair-0 feature tiles first so the pair-1
    # AllGather of the final slice can still be in flight
    K_ORDER = ([2 * g for g in range(GROUPS)]
               + [2 * g + 1 for g in range(GROUPS)])

    def proj(jj):
        """Output projection for token slice jj (our 256 feature rows)."""
        sl = slice(jj * 512, (jj + 1) * 512)
        for m in range(2):
            ps = ps_u if m == 0 else ps_w
            for ki, k in enumerate(K_ORDER):
                nc.tensor.matmul(
                    ps[:],
                    wproj_sb[k][:, m * 128:(m + 1) * 128],
                    yall_sb[k][:, sl],
                    start=(ki == 0), stop=(ki == KT - 1))
            o_m = opool.tile([128, 512], F32, tag="oproj")
            nc.vector.scalar_tensor_tensor(
                o_m[:], ps[:], bpr_sb[m][:, 0:1], rmask_sb[m][:, sl],
                ALU.add, ALU.mult)
            nc.scalar.dma_start(out[m * 128:(m + 1) * 128, sl], o_m[:])

    for n in range(NT512):
        qkv(n)
        attn(n)
        if n >= 2:
            gather_reads(n - 2)
            proj(n - 2)
    for j in (NT512 - 2, NT512 - 1):
        gather_reads(j)
        proj(j)

    ctx.close()


def prep_inputs(x, Wqkv, bqkv, Wproj, bproj, attn_drop_mask, resid_drop_mask):
    """Shard + lay out the full inputs for the 8 cores."""
    x = np.asarray(x, np.float32)
    Wqkv = np.asarray(Wqkv, np.float32)
    bqkv = np.asarray(bqkv, np.float32)
    Wproj = np.asarray(Wproj, np.float32)
    bproj = np.asarray(bproj, np.float32)
    attn_drop_mask = np.asarray(attn_drop_mask, bool)
    resid_drop_mask = np.asarray(resid_drop_mask, bool)

    tril = np.tril(np.ones((T, T), dtype=bool))
    triu128 = np.triu(np.ones((128, 128), np.float16))
    qscale = np.float32(1.0 / np.sqrt(HD))
    in_maps = []
    for core in range(N_CORES):
        b, g = divmod(core, GROUPS)
        cs = slice(g * 256, (g + 1) * 256)  # this group's feature rows
        wq_c = Wqkv[:, 0:1024][:, cs] * qscale  # fold 1/sqrt(hd) into q
        wk_c = Wqkv[:, 1024:2048][:, cs]
        wqk_c = np.concatenate([wq_c, wk_c], axis=1).astype(np.float16)
        wv_c = np.ascontiguousarray(Wqkv[:, 2048:3072][:, cs]).astype(np.float16)
        bq = (bqkv[0:1024][cs] * qscale).astype(np.float32)
        bk = bqkv[1024:2048][cs]
        bv = bqkv[2048:3072][cs]
        bqk_c = np.stack([bq[0:128], bq[128:256], bk[0:128], bk[128:256]])
        bqk_c = bqk_c.reshape(4, 128, 1).astype(np.float32)
        vbias_c = np.broadcast_to(bv, (128, 256)).astype(np.float32).copy()
        # combined causal & dropout mask, transposed to [tk, tq], {0,1}
        m = attn_drop_mask[b, g * HPG:(g + 1) * HPG] & tril
        maskT_c = np.ascontiguousarray(
            m.transpose(0, 2, 1)).astype(np.float16)
        rmaskT_c = np.ascontiguousarray(
            resid_drop_mask[b, :, cs].T).astype(np.float16)
        # fold the residual-dropout 1/KEEP into Wproj and bproj
        wproj_c = (Wproj[:, cs] / np.float32(KEEP)).astype(np.float16)
        bpr_c = (bproj[cs] / np.float32(KEEP)).astype(np.float32)
        bpr_c = bpr_c.reshape(2, 128, 1)
        in_maps.append(dict(
            xT=np.ascontiguousarray(x[b].T).astype(np.float16),
            wqk=np.ascontiguousarray(wqk_c),
            wv=wv_c,
            vbias=vbias_c,
            bqk=bqk_c,
            wproj=np.ascontiguousarray(wproj_c),
            bpr=bpr_c,
            maskT=maskT_c,
            rmaskT=rmaskT_c,
            triu=triu128,
        ))
    return in_maps


_NC_CACHE = {}


def _get_nc():
    if "nc" not in _NC_CACHE:
        _NC_CACHE["nc"] = build_kernel()
    return _NC_CACHE["nc"]


def kernel(trace=False, **inputs):
    nc = _get_nc()
    in_maps = prep_inputs(**inputs)
    res = run_bass_kernel_spmd(nc, in_maps, core_ids=list(range(N_CORES)),
                               trace=trace)
    y = np.empty((B, T, C), np.float32)
    for core in range(N_CORES):
        b, g = divmod(core, GROUPS)
        y[b, :, g * 256:(g + 1) * 256] = res.results[core]["out"].T
    kernel.last_result = res
    return y
